# revision 15
# baseline (speedup 1.0000x reference)
"""DHN pairwise-loss kernel for Trainium2 (Bass/Tile), 8-core SPMD.

Math (reference, per row i of sim = 0.5*b@b.T, pos = same-label mask):
    t[p,n]   = theta[p] - theta[n] - ALPHA          (clip is a no-op here; see below)
    val[p,n] = log1p(exp(t)) - t
    row_loss = sum over (p in pos, n in ~pos) val / (n_pos*n_neg)
    loss1    = mean(row_loss); loss2 = mean((b - sign(b))^2); total = loss1 + loss2

Device mapping (per core: 256 rows as 2 chunks of 128 partitions):
    val = l - t with l = ln(1 + u_p * v_j), u_p = e^{theta_p - ALPHA}, v_j = e^{-theta_j}.
    exp() factorizes, so per row-chunk:  v = Exp(-sim_chunk) (from PE matmul PSUM),
    then per positive-slot p ONE scalar-engine instruction
        Ln(u[:,p] * v + 1)  with per-partition scale=u[:,p], bias=1, accum_out=row-sum
    gives sum_j l for 128 rows at once.  Positive-positive pairs are removed by an
    identical (tiny) stream over the gathered positive thetas, and sum(t) over real
    pairs is computed analytically on host (fp64) and subtracted.  Row pads use
    +/-87 so exp() underflows and contributes exactly ln(1)=0.

    The fp32 clip(-100,50) in the reference is inactive for real pairs except
    t>50 cases whose val is exactly 0.0 in fp32 both ways.

Host does: sharding prep (per-row positive gathers = tiny subset of sim, exact
fp64 t-sums, 1/npairs weights) and the final 8-way scalar psum.
"""

import os
import numpy as np

N = 2048
D = 64
ALPHA = 5.0
LAMBDA = 1.0
NCORES = 8
# ln(1 + u*v) = ln(u) + ln(v + 1/u) keeps every Ln argument within the HW
# spline's accurate range [~2.5e-19, 2^64] (measured: garbage above 2^64,
# soft floor -45.86 below ~1e-20; naive u*v reaches e^77).  The ln(u) terms
# are folded into the per-row host constant K.
PAD_A = 43.0  # abias pad: uinv = e^-43 = 2.1e-19, in-range; Ln gives -43.0
PAD_P = 87.0  # apos pad:  vpos = e^-87 = 1.6e-38, vanishes next to any uinv
C_PAD = 43.0  # pad-p x pad-j element value is exactly -C_PAD

LAST_RESULTS = None  # BassKernelResults of the most recent run (for test harness)

_CACHE = {}


def _host_prep(b, y):
    """Partition rows into 8 cores x 2 chunk-slots and build per-core inputs."""
    b = np.ascontiguousarray(np.asarray(b, dtype=np.float32))
    y = np.asarray(y, dtype=np.int64).ravel()
    n = b.shape[0]
    assert b.shape == (N, D) and y.shape == (N,), (b.shape, y.shape)

    b64 = b.astype(np.float64)
    labels, inv, counts = np.unique(y, return_inverse=True, return_counts=True)
    n_row = counts[inv]  # positives count per row (includes self)

    # rows sorted by positive-count desc; slot0 = first half (big classes)
    order = np.argsort(-n_row, kind="stable")
    slot_rows = [order[: n // 2], order[n // 2:]]
    P0 = int(n_row[slot_rows[0]].max())
    P1 = int(n_row[slot_rows[1]].max())
    PT = P0 + P1

    # per-class data
    cls_idx = [np.nonzero(inv == c)[0] for c in range(len(labels))]
    cls_sum = np.stack([b64[ix].sum(axis=0) for ix in cls_idx])  # [C, D]
    all_sum = b64.sum(axis=0)

    # exact per-row quantities (fp64)
    s_pos = 0.5 * (b64 * cls_sum[inv]).sum(axis=1)  # sum of positive thetas
    s_all = 0.5 * (b64 @ all_sum)                   # sum of all thetas
    nc_r = n_row.astype(np.float64)
    npairs = nc_r * (n - nc_r)
    valid = (n_row >= 1) & (n_row < n)
    cnt = int(valid.sum())
    wvec_all = np.where(valid, 1.0 / np.maximum(npairs, 1.0) / max(cnt, 1), 0.0)

    # per-row positive thetas 0.5*<b_i, b_p>, grouped by class (fp64 -> f32)
    pos_theta = [None] * n
    for ix in cls_idx:
        g = 0.5 * (b64[ix] @ b64[ix].T)
        for k, r in enumerate(ix):
            pos_theta[r] = g[k]

    bth = np.ascontiguousarray(0.5 * b.T.astype(np.float32))  # [D, N] shared

    in_maps = []
    for core in range(NCORES):
        chunks = [slot_rows[0][core * 128:(core + 1) * 128],
                  slot_rows[1][core * 128:(core + 1) * 128]]
        rows = np.concatenate(chunks)
        brt = np.ascontiguousarray(b[rows].T)  # [D, 256]
        abias = np.full((128, PT), PAD_A, dtype=np.float32)
        apos = np.full((128, PT), PAD_P, dtype=np.float32)
        tw = np.zeros((128, 4), dtype=np.float32)
        for s, (off, Ps, chunk) in enumerate(zip((0, P0), (P0, P1), chunks)):
            for p, r in enumerate(chunk):
                th = pos_theta[r]
                abias[p, off:off + th.size] = th - ALPHA
                apos[p, off:off + th.size] = th
                ncr = nc_r[r]
                npad = Ps - ncr
                tw[p, 2 * s] = -(Ps * s_all[r] - (Ps + npad) * s_pos[r]
                                 + ncr * npad * ALPHA - npad * npad * C_PAD)
            tw[:, 2 * s + 1] = wvec_all[chunk]
        in_maps.append({
            "brt": brt, "bth": bth, "abias": abias, "apos": apos, "tw": tw,
        })
    return in_maps, P0, P1


def _build_bass(P0, P1):
    import concourse.bacc as bacc
    import concourse.tile as tile
    from concourse import mybir

    f32 = mybir.dt.float32
    AF = mybir.ActivationFunctionType
    PT = P0 + P1

    nc = bacc.Bacc("TRN2", target_bir_lowering=False, debug=False,
                   num_devices=NCORES)
    brt_d = nc.dram_tensor("brt", [D, 256], f32, kind="ExternalInput")
    bth_d = nc.dram_tensor("bth", [D, N], f32, kind="ExternalInput")
    ab_d = nc.dram_tensor("abias", [128, PT], f32, kind="ExternalInput")
    ap_d = nc.dram_tensor("apos", [128, PT], f32, kind="ExternalInput")
    tw_d = nc.dram_tensor("tw", [128, 4], f32, kind="ExternalInput")
    out_d = nc.dram_tensor("out", [1, 2], f32, kind="ExternalOutput")

    with tile.TileContext(nc) as tc:
        with (
            tc.tile_pool(name="const", bufs=1) as cpool,
            tc.tile_pool(name="v", bufs=2) as vpool,
            tc.tile_pool(name="scratch", bufs=2) as spool,
            tc.tile_pool(name="small", bufs=2) as mpool,
            tc.tile_pool(name="psum", bufs=2, space="PSUM") as ppool,
            tc.tile_pool(name="psum1", bufs=1, space="PSUM") as ppool1,
        ):
            brt = cpool.tile([D, 256], f32)
            nc.sync.dma_start(out=brt[:], in_=brt_d[:])
            bth = cpool.tile([D, N], f32)
            nc.sync.dma_start(out=bth[:], in_=bth_d[:])
            abias = cpool.tile([128, PT], f32)
            nc.sync.dma_start(out=abias[:], in_=ab_d[:])
            apos = cpool.tile([128, PT], f32)
            nc.sync.dma_start(out=apos[:], in_=ap_d[:])
            tw = cpool.tile([128, 4], f32)
            nc.sync.dma_start(out=tw[:], in_=tw_d[:])

            ones = cpool.tile([128, 1], f32)
            nc.vector.memset(ones[:], 1.0)
            negone = cpool.tile([128, 1], f32)
            nc.vector.memset(negone[:], -1.0)

            # uinv = e^{-(theta_pos-alpha)}, vpos = e^{-theta_pos}
            uinv = cpool.tile([128, PT], f32)
            nc.scalar.activation(out=uinv[:], in_=abias[:], func=AF.Exp, scale=-1.0)
            vpos = cpool.tile([128, PT], f32)
            nc.scalar.activation(out=vpos[:], in_=apos[:], func=AF.Exp, scale=-1.0)

            part_sums = []
            for s, (off, Ps) in enumerate(((0, P0), (P0, P1))):
                # v = e^{-sim} for this chunk's 128 rows, straight from PSUM
                v = vpool.tile([128, N], f32, tag="v")
                for q in range(N // 512):
                    pt = ppool.tile([128, 512], f32, tag="mm")
                    nc.tensor.matmul(pt[:], brt[:, s * 128:(s + 1) * 128],
                                     bth[:, q * 512:(q + 1) * 512],
                                     start=True, stop=True)
                    nc.scalar.activation(out=v[:, q * 512:(q + 1) * 512],
                                         in_=pt[:], func=AF.Exp, scale=-1.0)

                lall = mpool.tile([128, Ps], f32, tag=f"lall{s}")
                lpos = mpool.tile([128, Ps], f32, tag=f"lpos{s}")
                for p in range(Ps):
                    ucol = uinv[:, off + p:off + p + 1]
                    big = spool.tile([128, N], f32, tag="big")
                    nc.scalar.activation(out=big[:], in_=v[:], func=AF.Ln,
                                         bias=ucol,
                                         accum_out=lall[:, p:p + 1])
                    sm = spool.tile([128, Ps], f32, tag="sm")
                    nc.scalar.activation(out=sm[:], in_=vpos[:, off:off + Ps],
                                         func=AF.Ln, bias=ucol,
                                         accum_out=lpos[:, p:p + 1])

                la = mpool.tile([128, 1], f32, tag=f"la{s}")
                nc.vector.tensor_reduce(out=la[:], in_=lall[:],
                                        axis=mybir.AxisListType.X,
                                        op=mybir.AluOpType.add)
                lp = mpool.tile([128, 1], f32, tag=f"lp{s}")
                nc.vector.tensor_reduce(out=lp[:], in_=lpos[:],
                                        axis=mybir.AxisListType.X,
                                        op=mybir.AluOpType.add)
                r1 = mpool.tile([128, 1], f32, tag=f"r1{s}")
                nc.vector.tensor_sub(out=r1[:], in0=la[:], in1=lp[:])
                r2 = mpool.tile([128, 1], f32, tag=f"r2{s}")
                nc.vector.tensor_sub(out=r2[:], in0=r1[:], in1=tw[:, 2 * s:2 * s + 1])
                r3 = mpool.tile([128, 1], f32, tag=f"r3{s}")
                nc.vector.tensor_mul(out=r3[:], in0=r2[:], in1=tw[:, 2 * s + 1:2 * s + 2])
                pr = ppool1.tile([1, 1], f32, tag=f"pr{s}")
                nc.tensor.matmul(pr[:], r3[:], ones[:], start=True, stop=True)
                sb = mpool.tile([1, 1], f32, tag=f"sb{s}")
                nc.vector.tensor_copy(out=sb[:], in_=pr[:])
                part_sums.append(sb)

            # loss2 partial: sum (|b|-1)^2 over this core's 256 rows
            absb = spool.tile([D, 256], f32, tag="absb")
            nc.scalar.activation(out=absb[:], in_=brt[:], func=AF.Abs)
            sq = spool.tile([D, 256], f32, tag="sq")
            qcol = mpool.tile([D, 1], f32, tag="qcol")
            nc.scalar.activation(out=sq[:], in_=absb[:], func=AF.Square,
                                 bias=negone[:D, :], accum_out=qcol[:])
            pq = ppool1.tile([1, 1], f32, tag="pq")
            nc.tensor.matmul(pq[:], qcol[:], ones[:D, :], start=True, stop=True)

            outs = cpool.tile([1, 2], f32)
            nc.vector.tensor_add(out=outs[0:1, 0:1], in0=part_sums[0][:],
                                 in1=part_sums[1][:])
            nc.vector.tensor_copy(out=outs[0:1, 1:2], in_=pq[:])
            nc.sync.dma_start(out=out_d[:], in_=outs[:])

    nc.finalize()
    return nc


def kernel(b, y):
    global LAST_RESULTS
    from concourse.bass_utils import run_bass_kernel_spmd

    in_maps, P0, P1 = _host_prep(b, y)

    key = (P0, P1)
    if key not in _CACHE:
        _CACHE[key] = _build_bass(P0, P1)
    nc = _CACHE[key]

    trace = bool(int(os.environ.get("BASS_DHN_TRACE", "0")))
    res = run_bass_kernel_spmd(nc, in_maps, core_ids=list(range(NCORES)),
                               trace=trace)
    LAST_RESULTS = res

    loss1 = np.float64(0.0)
    loss2_sum = np.float64(0.0)
    for r in res.results:
        o = r["out"]
        loss1 += np.float64(o[0, 0])
        loss2_sum += np.float64(o[0, 1])
    loss2 = loss2_sum / (N * D)
    total = loss1 + LAMBDA * loss2
    return (np.float32(total), np.float32(loss1), np.float32(loss2))


# revision 16
# speedup vs baseline: 1.2035x; 1.2035x over previous
"""DHN pairwise-loss kernel for Trainium2 (Bass/Tile), 8-core SPMD.

Math (reference, per row i of sim = 0.5*b@b.T, pos = same-label mask):
    t[p,n]   = theta[p] - theta[n] - ALPHA          (fp32 clip is a no-op here)
    val[p,n] = log1p(exp(t)) - t
    row_loss = sum over (p in pos, n in ~pos) val / (n_pos*n_neg)
    loss1    = mean(row_loss); loss2 = mean((b - sign(b))^2); total = loss1 + loss2

Device mapping (per core: 256 rows as 2 chunks of 128 partitions):
    val = l - t with l = ln(1 + u_p * v_j), u_p = e^{theta_p - ALPHA}, v_j = e^{-theta_j};
    sum(t) over real pairs is analytic on host (fp64).  exp() factorizes, so per
    row-chunk v = Exp(-sim') straight out of the PE matmul PSUM, where
        sim' = 0.5*b@b.T + MASKC*Y@Y.T     (Y = one-hot labels, fused into the
    matmul as 32 extra contraction rows) pushes same-label pairs to ~theta+100,
    so their v underflows to exactly 0 and positive-positive pairs drop out
    analytically: Ln(0 + uinv_p) = -B_p, folded into the host constant K.
    Then ONE scalar-engine instruction per positive-slot p
        Ln(v + uinv[:,p])  with per-partition bias, accum_out = row-sum
    covers 128 rows x 2048 pairs.  HW-measured Ln is accurate on
    [~2.5e-19, 2^64] and garbage outside, hence the ln(u)+ln(v+1/u) split
    (naive u*v reaches e^77) and the e^-43 slot padding.

Host does: sharding prep (tiny per-row positive-theta gathers, exact fp64
constants, 1/npairs weights) and the final 8-way scalar psum.
"""

import os
import numpy as np

N = 2048
D = 64
ALPHA = 5.0
LAMBDA = 1.0
NCORES = 8
PAD_A = 43.0   # abias pad: uinv = e^-43 = 2.1e-19, in Ln's accurate range
C_PAD = 43.0   # Ln(~0 + e^-43) = -43.0 exactly (HW-verified)
MASKC = 100.0  # same-label sim offset: v = e^-(theta+100) underflows to 0

LAST_RESULTS = None  # BassKernelResults of the most recent run (for test harness)

_CACHE = {}


def _host_prep(b, y):
    """Partition rows into 8 cores x 2 chunk-slots and build per-core inputs."""
    b = np.ascontiguousarray(np.asarray(b, dtype=np.float32))
    y = np.asarray(y, dtype=np.int64).ravel()
    n = b.shape[0]
    assert b.shape == (N, D) and y.shape == (N,), (b.shape, y.shape)

    b64 = b.astype(np.float64)
    labels, inv, counts = np.unique(y, return_inverse=True, return_counts=True)
    ncls = len(labels)
    n_row = counts[inv]  # positives count per row (includes self)

    # rows sorted by positive-count desc; slot0 = first half (big classes)
    order = np.argsort(-n_row, kind="stable")
    slot_rows = [order[: n // 2], order[n // 2:]]
    P0 = int(n_row[slot_rows[0]].max())
    P1 = int(n_row[slot_rows[1]].max())

    # per-class data
    cls_idx = [np.nonzero(inv == c)[0] for c in range(ncls)]
    cls_sum = np.stack([b64[ix].sum(axis=0) for ix in cls_idx])  # [C, D]
    all_sum = b64.sum(axis=0)

    # exact per-row quantities (fp64)
    s_pos = 0.5 * (b64 * cls_sum[inv]).sum(axis=1)  # sum of positive thetas
    s_all = 0.5 * (b64 @ all_sum)                   # sum of all thetas
    nc_r = n_row.astype(np.float64)
    npairs = nc_r * (n - nc_r)
    valid = (n_row >= 1) & (n_row < n)
    cnt = int(valid.sum())
    wvec_all = np.where(valid, 1.0 / np.maximum(npairs, 1.0) / max(cnt, 1), 0.0)

    # per-row positive thetas 0.5*<b_i, b_p>, grouped by class (fp64 -> f32)
    pos_theta = [None] * n
    for ix in cls_idx:
        g = 0.5 * (b64[ix] @ b64[ix].T)
        for k, r in enumerate(ix):
            pos_theta[r] = g[k]

    onehot = np.zeros((n, ncls), dtype=np.float32)
    onehot[np.arange(n), inv] = 1.0
    bth = np.concatenate([0.5 * b.T, onehot.T], axis=0)      # [D+C, N] shared
    bth = np.ascontiguousarray(bth.astype(np.float32))

    in_maps = []
    for core in range(NCORES):
        chunks = [slot_rows[0][core * 128:(core + 1) * 128],
                  slot_rows[1][core * 128:(core + 1) * 128]]
        rows = np.concatenate(chunks)
        brt = np.concatenate([b[rows].T, MASKC * onehot[rows].T], axis=0)
        brt = np.ascontiguousarray(brt.astype(np.float32))   # [D+C, 256]
        abias = np.full((128, P0 + P1), PAD_A, dtype=np.float32)
        tw = np.zeros((128, 4), dtype=np.float32)
        for s, (off, Ps, chunk) in enumerate(
                zip((0, P0), (P0, P1), chunks)):
            for p, r in enumerate(chunk):
                th = pos_theta[r]
                abias[p, off:off + th.size] = th - ALPHA
                ncr = nc_r[r]
                npad = Ps - ncr
                # row_val = Dall + K;  tw0 = -K so device does (Dall - tw0)*tw1
                K = (Ps * s_all[r] - npad * s_pos[r]
                     + C_PAD * ncr * npad - ncr * ncr * ALPHA)
                tw[p, 2 * s] = -K
            tw[:, 2 * s + 1] = wvec_all[chunk]
        in_maps.append({"brt": brt, "bth": bth, "abias": abias, "tw": tw})
    return in_maps, P0, P1, ncls


def _build_bass(P0, P1, ncls):
    import concourse.bacc as bacc
    import concourse.tile as tile
    from concourse import mybir

    f32 = mybir.dt.float32
    AF = mybir.ActivationFunctionType
    PT = P0 + P1
    KD = D + ncls

    nc = bacc.Bacc("TRN2", target_bir_lowering=False, debug=False,
                   num_devices=NCORES)
    brt_d = nc.dram_tensor("brt", [KD, 256], f32, kind="ExternalInput")
    bth_d = nc.dram_tensor("bth", [KD, N], f32, kind="ExternalInput")
    ab_d = nc.dram_tensor("abias", [128, PT], f32, kind="ExternalInput")
    tw_d = nc.dram_tensor("tw", [128, 4], f32, kind="ExternalInput")
    out_d = nc.dram_tensor("out", [1, 2], f32, kind="ExternalOutput")

    with tile.TileContext(nc) as tc:
        with (
            tc.tile_pool(name="const", bufs=1) as cpool,
            tc.tile_pool(name="scratch", bufs=3) as spool,
            tc.tile_pool(name="small", bufs=2) as mpool,
            tc.tile_pool(name="psum", bufs=2, space="PSUM") as ppool,
            tc.tile_pool(name="psum1", bufs=1, space="PSUM") as ppool1,
        ):
            brt = cpool.tile([KD, 256], f32)
            nc.sync.dma_start(out=brt[:], in_=brt_d[:])
            bth = cpool.tile([KD, N], f32)
            nc.sync.dma_start(out=bth[:], in_=bth_d[:])
            abias = cpool.tile([128, PT], f32)
            nc.sync.dma_start(out=abias[:], in_=ab_d[:])
            tw = cpool.tile([128, 4], f32)
            nc.sync.dma_start(out=tw[:], in_=tw_d[:])

            ones = cpool.tile([128, 1], f32)
            nc.vector.memset(ones[:], 1.0)

            # all Exp work first (one ACT table in play at a time)
            uinv = cpool.tile([128, PT], f32)
            nc.scalar.activation(out=uinv[:], in_=abias[:], func=AF.Exp,
                                 scale=-1.0)
            vs = []
            for s in range(2):
                v = cpool.tile([128, N], f32, tag=f"v{s}")
                for q in range(N // 512):
                    pt = ppool.tile([128, 512], f32, tag="mm")
                    nc.tensor.matmul(pt[:], brt[:, s * 128:(s + 1) * 128],
                                     bth[:, q * 512:(q + 1) * 512],
                                     start=True, stop=True)
                    nc.scalar.activation(out=v[:, q * 512:(q + 1) * 512],
                                         in_=pt[:], func=AF.Exp, scale=-1.0)
                vs.append(v)

            # loss2 on the idle Vector engine: sum (|b|-1)^2 over 256 rows
            bb = brt[:D, :]
            nb = mpool.tile([D, 256], f32, tag="nb")
            nc.vector.tensor_scalar_mul(nb[:], bb, -1.0)
            ab = mpool.tile([D, 256], f32, tag="ab")
            nc.vector.tensor_max(ab[:], bb, nb[:])
            nc.vector.tensor_scalar_add(ab[:], ab[:], -1.0)
            sq = mpool.tile([D, 256], f32, tag="sq")
            nc.vector.tensor_mul(sq[:], ab[:], ab[:])
            qcol = mpool.tile([D, 1], f32, tag="qcol")
            nc.vector.tensor_reduce(out=qcol[:], in_=sq[:],
                                    axis=mybir.AxisListType.X,
                                    op=mybir.AluOpType.add)
            pq = ppool1.tile([1, 1], f32, tag="pq")
            nc.tensor.matmul(pq[:], qcol[:], ones[:D, :], start=True, stop=True)

            # Ln streams
            part_sums = []
            for s, (off, Ps) in enumerate(((0, P0), (P0, P1))):
                lall = mpool.tile([128, Ps], f32, tag=f"lall{s}")
                for p in range(Ps):
                    big = spool.tile([128, N], f32, tag="big")
                    nc.scalar.activation(out=big[:], in_=vs[s][:], func=AF.Ln,
                                         bias=uinv[:, off + p:off + p + 1],
                                         accum_out=lall[:, p:p + 1])
                la = mpool.tile([128, 1], f32, tag=f"la{s}")
                nc.vector.tensor_reduce(out=la[:], in_=lall[:],
                                        axis=mybir.AxisListType.X,
                                        op=mybir.AluOpType.add)
                r2 = mpool.tile([128, 1], f32, tag=f"r2{s}")
                nc.vector.tensor_sub(out=r2[:], in0=la[:],
                                     in1=tw[:, 2 * s:2 * s + 1])
                r3 = mpool.tile([128, 1], f32, tag=f"r3{s}")
                nc.vector.tensor_mul(out=r3[:], in0=r2[:],
                                     in1=tw[:, 2 * s + 1:2 * s + 2])
                pr = ppool1.tile([1, 1], f32, tag=f"pr{s}")
                nc.tensor.matmul(pr[:], r3[:], ones[:], start=True, stop=True)
                sb = mpool.tile([1, 1], f32, tag=f"sb{s}")
                nc.vector.tensor_copy(out=sb[:], in_=pr[:])
                part_sums.append(sb)

            outs = cpool.tile([1, 2], f32)
            nc.vector.tensor_add(out=outs[0:1, 0:1], in0=part_sums[0][:],
                                 in1=part_sums[1][:])
            nc.vector.tensor_copy(out=outs[0:1, 1:2], in_=pq[:])
            nc.sync.dma_start(out=out_d[:], in_=outs[:])

    nc.finalize()
    return nc


def kernel(b, y):
    global LAST_RESULTS
    from concourse.bass_utils import run_bass_kernel_spmd

    in_maps, P0, P1, ncls = _host_prep(b, y)

    key = (P0, P1, ncls)
    if key not in _CACHE:
        _CACHE[key] = _build_bass(P0, P1, ncls)
    nc = _CACHE[key]

    trace = bool(int(os.environ.get("BASS_DHN_TRACE", "0")))
    res = run_bass_kernel_spmd(nc, in_maps, core_ids=list(range(NCORES)),
                               trace=trace)
    LAST_RESULTS = res

    loss1 = np.float64(0.0)
    loss2_sum = np.float64(0.0)
    for r in res.results:
        o = r["out"]
        loss1 += np.float64(o[0, 0])
        loss2_sum += np.float64(o[0, 1])
    loss2 = loss2_sum / (N * D)
    total = loss1 + LAMBDA * loss2
    return (np.float32(total), np.float32(loss1), np.float32(loss2))
